# revision 9
# baseline (speedup 1.0000x reference)
"""ContextualLoss forward on 8 trn2 NeuronCores — single-matmul-pass version.

Problem: X, Y [4, 256, 64, 64] f32 ->  loss [4] f32
  y_mean[c] = mean_hw(Y);  Xc = X - y_mean; Yc = Y - y_mean
  Xn, Yn: L2-normalized over C per spatial position; S = Xn^T @ Yn  [N, N]
  d = 1 - S; dmin = row min d; w = exp((1 - d/(dmin+1e-3))/0.1); A = w/rowsum(w)
  loss_b = -log(mean_n max_m A[n, m])

Algebra (per row n, g = 1/||Xc_n||, S = Xc^T @ Yn with Yn = Yc*invnY):
  max_m A[n,:] = 1 / Z'[n],   Z'[n] = sum_m exp((S[n,m] - smax[n]) * s[n])
  smax = row max S,  ndm = 1.001 - smax*g  (= dmin + 1e-3),  s = 10*g/ndm.
(The softmax ratio is invariant to common additive shifts in the exponent,
so the exact d/dmin form reduces to this shifted-scaled one.)

Engine plan per core (4 samples x 2 row-halves across 8 cores):
  PE    single bf16 matmul pass -> PSUM [128,2048] halves (2 bufs = 8 banks),
        matmuls grouped by stationary operand (2 LDWEIGHTS per half).
  DVE   row max from PSUM (FD=2048) + copies cols [0,CD) of each half to the
        SBUF f32 slab + the per-block reciprocal.
  ACT   copies cols [CD,2048) of each half PSUM->SBUF (Identity), per-block
        ndm, then one Exp over the [128,4096] slab with per-row scale/bias
        and accum_out = Z'.  All funcs in one table set (natural_log_exp).
  Pool  invnY broadcast + the three per-block [128,1] multiplies.
Host combines: loss_b = -log((sum of cores' [128,1] outputs)/4096).
"""

import numpy as np

B, C, HW = 4, 256, 4096
HALF = HW // 2
NCORES = 8
NB = HALF // 128      # 16 row blocks per core
H_INV = 10.0          # 1/h with h = 0.1
CD = 600              # columns per half copied by DVE (rest by ACT)

_nc_cache = None


def _build():
    import concourse.bass as bass
    import concourse.bacc as bacc
    import concourse.tile as tile
    from concourse import mybir

    f32 = mybir.dt.float32
    bf16 = mybir.dt.bfloat16
    AF = mybir.ActivationFunctionType
    OP = mybir.AluOpType
    AX = mybir.AxisListType

    nc = bacc.Bacc(None)

    y_dram = nc.dram_tensor("y", [C, HW], f32, kind="ExternalInput")
    x_dram = nc.dram_tensor("xh", [C, HALF], f32, kind="ExternalInput")
    out_dram = nc.dram_tensor("out", [128, 1], f32, kind="ExternalOutput")
    bounce = nc.dram_tensor("bounce", [1, HW], f32)  # invnY transpose bounce

    with tile.TileContext(nc) as tc:
        with (
            tc.tile_pool(name="persist", bufs=1) as P,
            tc.tile_pool(name="stats", bufs=3) as ST,
        ):
            # ---------------- constants / persistent tiles ----------------
            ones_mm = P.tile([128, 1], bf16)
            nc.vector.memset(ones_mm, 1.0)
            negones = P.tile([128, 1], f32)
            nc.vector.memset(negones, -1.0)
            c1001 = P.tile([128, 1], f32)
            nc.vector.memset(c1001, 1.001)

            yn = [P.tile([128, HW], bf16, tag=f"yn{i}", name=f"yn{i}") for i in range(2)]
            xcb = [P.tile([128, HALF], bf16, tag=f"xcb{i}", name=f"xcb{i}") for i in range(2)]
            g10 = P.tile([128, NB], f32, tag="g10")      # 10 * invnX
            gneg = P.tile([128, NB], f32, tag="gneg")    # -invnX
            zall = P.tile([128, NB], f32, tag="zall")
            negmean = [P.tile([128, 1], f32, tag=f"nm{i}", name=f"nm{i}") for i in range(2)]

            # ---------------- setup (freed before main loop) ----------------
            with (
                tc.tile_pool(name="setup", bufs=1) as SU,
                tc.tile_pool(name="sups", bufs=1, space="PSUM") as SUPS,
            ):
                y_sb = [SU.tile([128, HW], f32, tag=f"y{i}", name=f"y{i}") for i in range(2)]
                x_sb = [SU.tile([128, HALF], f32, tag=f"x{i}", name=f"x{i}") for i in range(2)]
                for i in range(2):
                    for ch in range(4):
                        sl = slice(ch * 1024, (ch + 1) * 1024)
                        nc.sync.dma_start(out=y_sb[i][:, sl], in_=y_dram[128 * i : 128 * (i + 1), sl])
                for i in range(2):
                    nc.sync.dma_start(out=x_sb[i], in_=x_dram[128 * i : 128 * (i + 1), :])

                # per-channel spatial mean of Y (chunked to pipeline behind DMA)
                ysp = SU.tile([128, 8], f32, tag="ysp")
                for i in range(2):
                    for ch in range(4):
                        sl = slice(ch * 1024, (ch + 1) * 1024)
                        nc.vector.reduce_sum(out=ysp[:, i * 4 + ch : i * 4 + ch + 1], in_=y_sb[i][:, sl], axis=AX.X)
                for i in range(2):
                    ys_i = SU.tile([128, 1], f32, tag=f"ys{i}", name=f"ys{i}")
                    nc.vector.reduce_sum(out=ys_i, in_=ysp[:, i * 4 : (i + 1) * 4], axis=AX.X)
                    nc.vector.tensor_scalar_mul(out=negmean[i], in0=ys_i, scalar1=-1.0 / HW)

                # squares of centered features (ACT, fused centering via bias)
                ysq = [SU.tile([128, HW], bf16, tag=f"ysq{i}", name=f"ysq{i}") for i in range(2)]
                xsq = [SU.tile([128, HALF], bf16, tag=f"xsq{i}", name=f"xsq{i}") for i in range(2)]
                for ch in range(2):
                    sl = slice(ch * 2048, (ch + 1) * 2048)
                    for i in range(2):
                        nc.scalar.activation(out=ysq[i][:, sl], in_=y_sb[i][:, sl], func=AF.Square, bias=negmean[i], scale=1.0)
                for i in range(2):
                    nc.scalar.activation(out=xsq[i], in_=x_sb[i], func=AF.Square, bias=negmean[i], scale=1.0)

                # centered X in bf16 (ACT)
                for i in range(2):
                    nc.scalar.activation(out=xcb[i], in_=x_sb[i], func=AF.Identity, bias=negmean[i], scale=1.0)

                # transposed sum-of-squares: matmul(lhsT=sq block, rhs=ones)
                # puts ss for positions pb*128..pb*128+127 into psum column pb.
                ssyT = SUPS.tile([128, 32], f32, tag="ssy")
                ssxT = SUPS.tile([128, 16], f32, tag="ssx")
                for pb in range(32):
                    psl = slice(pb * 128, (pb + 1) * 128)
                    nc.tensor.matmul(ssyT[:, pb : pb + 1], ysq[0][:, psl], ones_mm, start=True, stop=False)
                    nc.tensor.matmul(ssyT[:, pb : pb + 1], ysq[1][:, psl], ones_mm, start=False, stop=True)
                for pb in range(16):
                    psl = slice(pb * 128, (pb + 1) * 128)
                    nc.tensor.matmul(ssxT[:, pb : pb + 1], xsq[0][:, psl], ones_mm, start=True, stop=False)
                    nc.tensor.matmul(ssxT[:, pb : pb + 1], xsq[1][:, psl], ones_mm, start=False, stop=True)

                # 1/norm = exp(-0.5*ln(ss)) on the [128, nb] transposed layout
                lny = SU.tile([128, 32], f32, tag="lny")
                nc.scalar.activation(out=lny, in_=ssyT, func=AF.Ln, bias=0.0, scale=1.0)
                invnyT = SU.tile([128, 32], f32, tag="invnyT")
                nc.scalar.activation(out=invnyT, in_=lny, func=AF.Exp, bias=0.0, scale=-0.5)
                lnx = SU.tile([128, 16], f32, tag="lnx")
                nc.scalar.activation(out=lnx, in_=ssxT, func=AF.Ln, bias=0.0, scale=1.0)
                invnxT = SU.tile([128, 16], f32, tag="invnxT")
                nc.scalar.activation(out=invnxT, in_=lnx, func=AF.Exp, bias=0.0, scale=-0.5)
                nc.vector.tensor_scalar_mul(out=g10, in0=invnxT, scalar1=H_INV)
                nc.vector.tensor_scalar_mul(out=gneg, in0=invnxT, scalar1=-1.0)

                # invnY column layout -> DRAM row -> broadcast to 128 partitions
                nc.gpsimd.dma_start(out=bounce.rearrange("o (j p) -> (o p) j", p=128), in_=invnyT)
                row = SU.tile([1, HW], f32, tag="row")
                nc.gpsimd.dma_start(out=row, in_=bounce[:, :])
                invny_b = SU.tile([128, HW], f32, tag="invnyb")
                for ch in range(8):
                    sl = slice(ch * 512, (ch + 1) * 512)
                    nc.gpsimd.partition_broadcast(invny_b[:, sl], row[0:1, sl])

                # yn = (Y - mean) * invnY  in bf16 (DVE, 2048-col chunks)
                for i in range(2):
                    for ch in range(2):
                        sl = slice(ch * 2048, (ch + 1) * 2048)
                        nc.vector.scalar_tensor_tensor(
                            out=yn[i][:, sl], in0=y_sb[i][:, sl], scalar=negmean[i],
                            in1=invny_b[:, sl], op0=OP.add, op1=OP.mult,
                        )

            # ---------------- main loop over 16 row blocks ----------------
            with (
                tc.tile_pool(name="ps", bufs=2, space="PSUM") as PS,
                tc.tile_pool(name="slab", bufs=3) as SL,
                tc.tile_pool(name="dumps", bufs=2) as DU,
            ):
                # Software-pipelined: block nb's exp is emitted during block
                # nb+1 (after its PSUM copies), so the stats chain latency
                # (max -> ndm -> rr -> Pool products) never blocks the bank
                # recycle path.  Engine queues per block:
                #   PE:   16 matmuls (2 LDW-grouped accum chains per half)
                #   DVE:  maxh0, copyD-h0, maxh1, copyD-h1, comb, ndm, rr
                #   ACT:  copyA-h0, copyA-h1, exp(prev block)
                #   Pool: scol, t0, bcol
                pend = None  # (slab, scol, bcol, nb) awaiting exp
                for nb in range(NB):
                    nsl = slice(nb * 128, (nb + 1) * 128)
                    slab = SL.tile([128, HW], f32, tag="s")
                    mx2 = ST.tile([128, 2], f32, tag="mx2")
                    smax = ST.tile([128, 1], f32, tag="smax")
                    for h in range(2):
                        ps_h = PS.tile([128, HALF], f32, tag="ps", name=f"ps{nb}_{h}")
                        for ci in range(2):
                            for cc in range(4):
                                csl = slice(cc * 512, (cc + 1) * 512)
                                msl = slice(h * HALF + cc * 512, h * HALF + (cc + 1) * 512)
                                nc.tensor.matmul(
                                    ps_h[:, csl], xcb[ci][:, nsl], yn[ci][:, msl],
                                    start=(ci == 0), stop=(ci == 1),
                                )
                        nc.vector.reduce_max(out=mx2[:, h : h + 1], in_=ps_h, axis=AX.X)
                        nc.vector.tensor_copy(slab[:, h * HALF : h * HALF + CD], ps_h[:, 0:CD])
                        nc.scalar.activation(
                            out=slab[:, h * HALF + CD : (h + 1) * HALF],
                            in_=ps_h[:, CD:HALF], func=AF.Identity, bias=0.0, scale=1.0,
                        )
                    nc.vector.reduce_max(out=smax, in_=mx2, axis=AX.X)
                    # stats all on DVE tail + Pool (exp consumes them a block later)
                    ndm = ST.tile([128, 1], f32, tag="ndm")
                    nc.vector.scalar_tensor_tensor(
                        out=ndm, in0=smax, scalar=gneg[:, nb : nb + 1], in1=c1001,
                        op0=OP.mult, op1=OP.add,
                    )
                    rr = ST.tile([128, 1], f32, tag="rr")
                    nc.vector.reciprocal(out=rr, in_=ndm)
                    scol = ST.tile([128, 1], f32, tag="sc")
                    nc.gpsimd.tensor_tensor(out=scol, in0=rr, in1=g10[:, nb : nb + 1], op=OP.mult)
                    t0 = ST.tile([128, 1], f32, tag="t0")
                    nc.gpsimd.tensor_tensor(out=t0, in0=smax, in1=scol, op=OP.mult)
                    bcol = ST.tile([128, 1], f32, tag="bc")
                    nc.gpsimd.tensor_tensor(out=bcol, in0=t0, in1=negones, op=OP.mult)
                    if pend is not None:
                        p_slab, p_scol, p_bcol, p_nb = pend
                        dump = DU.tile([128, HW], bf16, tag="d")
                        nc.scalar.activation(
                            out=dump, in_=p_slab, func=AF.Exp,
                            bias=p_bcol, scale=p_scol,
                            accum_out=zall[:, p_nb : p_nb + 1],
                        )
                    pend = (slab, scol, bcol, nb)
                p_slab, p_scol, p_bcol, p_nb = pend
                dump = DU.tile([128, HW], bf16, tag="d")
                nc.scalar.activation(
                    out=dump, in_=p_slab, func=AF.Exp,
                    bias=p_bcol, scale=p_scol,
                    accum_out=zall[:, p_nb : p_nb + 1],
                )

                # ---------------- epilogue: sum_n 1/Z ----------------
                rz = P.tile([128, NB], f32, tag="rz")
                nc.vector.reciprocal(out=rz, in_=zall)
                acc = P.tile([128, 1], f32, tag="acc")
                nc.vector.reduce_sum(out=acc, in_=rz, axis=AX.X)
                nc.gpsimd.dma_start(out=out_dram[:, :], in_=acc)

    nc.finalize()
    return nc


def _get_nc():
    global _nc_cache
    if _nc_cache is None:
        _nc_cache = _build()
    return _nc_cache


def run_cores(inputs, **kwargs):
    """Run the 8-core SPMD kernel; returns (loss[4], BassKernelResults)."""
    from concourse.bass_utils import run_bass_kernel_spmd

    nc = _get_nc()
    X = np.asarray(inputs["X_features"], dtype=np.float32).reshape(B, C, HW)
    Y = np.asarray(inputs["Y_features"], dtype=np.float32).reshape(B, C, HW)
    in_maps = []
    for core in range(NCORES):
        b, h = divmod(core, 2)
        in_maps.append(
            {
                "y": np.ascontiguousarray(Y[b]),
                "xh": np.ascontiguousarray(X[b, :, h * HALF : (h + 1) * HALF]),
            }
        )
    res = run_bass_kernel_spmd(nc, in_maps, core_ids=list(range(NCORES)), **kwargs)
    acc = np.stack(
        [res.results[i]["out"].reshape(-1).astype(np.float64) for i in range(NCORES)]
    )  # [8, 128]
    cx = acc.reshape(B, 2 * 128).sum(axis=1) / HW
    loss = (-np.log(cx)).astype(np.float32)
    return loss, res


def kernel(**inputs):
    return run_cores(inputs)[0]
